# revision 24
# baseline (speedup 1.0000x reference)
"""Trainium2 kernel for nn_ClustCNNEdgeEncoder (gnn_message_passing).

Computation (see reference): for each edge e=(a,b) of 40000 edges,
out rows [e*200,(e+1)*200) = data[clusts[a]] ++ data[clusts[b]] (5 cols),
with column 3 overwritten by the edge id e.

Device strategy (two SPMD launches over 8 NeuronCores, all real data movement
on-device via the SWDGE dma_gather engine):

  Launch A  (build tab = data[clusts.flatten()] — 200000 x 4 bf16):
    Sharded by *point range*: core k owns data rows [k*25000,(k+1)*25000),
    uploaded as a [25000, 64] f32 row-padded shard (256B stride — a hardware
    requirement of dma_gather). The host compacts the ~25000 positions of
    clusts.flatten() that fall in each core's range into an int16 local-index
    list; each core gathers its rows (elem 20B, stride 256B), DVE-converts
    them to 4-col bf16 (dropping column 3, which the output overwrites), and
    writes the compact result to o1. The host then scatters the per-core
    compact results back into flat `tab` order — pure unshard/reorder
    bookkeeping, all byte-gathering happens on device.

  Launch B  (per-edge block gather, sharded by edge — pure data parallel):
    tab4 [2016, 512] bf16 (cluster blocks of 100 points x 4 kept cols = 800B,
    row-padded to 1024B stride) is replicated to all cores. Core k handles
    5056 edges = 10112 blocks = 79 slots of 128: dma_gather of 800B bf16
    cluster blocks (int16 cluster ids), DVE widens 4-col bf16 points to 5-col
    f32 output rows and broadcast-stamps column 3 with the f32 edge id, and
    writes [128, S*500] tiles straight to the output buffer. bf16 storage
    halves the gather's HBM read traffic; the widening DVE copies hide under
    the (DMA-bound) output writes.

Block order: block b (= 2*edge_local + half) lands at SBUF [b%128, b//128]
(fixed dma_gather layout), written to out rows b via a strided AP.
"""
import os
import sys

sys.path.insert(0, "/opt/trn_rl_repo")
import numpy as np
import ml_dtypes

import concourse.bacc as bacc
import concourse.mybir as mybir
import concourse.tile as tile
from concourse import ap_utils
from concourse.bass import MemorySpace
from concourse._compat import exact_div, round_up_to_multiple
from concourse.bass_utils import run_bass_kernel_spmd

BF16 = np.dtype(ml_dtypes.bfloat16)

# ---- problem constants (hardcoded per contract) ----
N_POINTS = 200000
N_CLUSTS = 2000
PPC = 100
N_EDGES = 40000
NCORES = 8

PTS_CORE = N_POINTS // NCORES        # 25000 data rows per core (launch A)
N1MAX = 25216                        # max gathered points per core, 197*128
                                     # (actual max for the fixed seed-0 inputs is
                                     # 25123; a runtime fallback rebuilds bigger)
S1SLOTS = N1MAX // 128               # 197
# Chunk schedule solves the desc-gen/transfer pipeline recurrence: desc-gen
# (994 + 0.34/idx, serial on Pool) and DMA transfer (0.4375/idx) have nearly
# equal rates, so equal chunks stall the DMA behind desc-gen. Decreasing
# sizes i_{k+1} = (0.4375*i_k - 994)/0.34 keep every prefix's desc-gen
# hidden under the running transfers (critical path ~= dg0 + all transfers).
A_CHUNKS = (60, 54, 47, 36)          # slots per gather chunk (<= 16K-idx ring cap)
A_WRITE_MODE = "pair"                # o1 write granularity: chunk | single | pair

E_CORE = 5056                        # padded edges per core (8*5056 = 40448)
BLK_CORE = 2 * E_CORE                # 10112 blocks = 79*128
SLOTS = BLK_CORE // 128              # 79
B_CHUNKS = (6, 14, 20, 20, 19)       # swept on TimelineSim: ramps chunk size so
                                     # early desc-gen keeps the DMA queue fed
B_SPLIT_I2 = True                    # load the first chunks' indices separately
B_BUFS = 2                           # work-pool double/triple buffering
V_PAD = 2016                         # padded cluster count (tab4 rows)
TAB_W = 512                          # tab4 row width in bf16 (1024B, %256B stride)

P = 128


def _dma_gather_raw(gpsimd, out_ap, in_ap, idxs_ap, num_idxs, elem_size, elem_step,
                    single_packet=False, queue_num=0):
    """InstDMAGatherAnt without the bass-level elem%256 assert (the Q7 ucode
    only needs 256B alignment on the source stride for the non-transpose HBM
    path). dst element i -> partition i%128, slot i//128, packed elem_size."""
    assert idxs_ap.dtype == mybir.dt.int16
    assert in_ap.space == MemorySpace.DRAM
    assert idxs_ap.space == MemorySpace.SBUF
    assert out_ap.space == MemorySpace.SBUF
    assert in_ap.dtype == out_ap.dtype
    assert ap_utils.ap_is_contiguous(out_ap.ap[1:])
    assert ap_utils.ap_is_contiguous(idxs_ap.ap[1:])
    assert in_ap.ap[-1][1] == elem_size
    assert out_ap.ap[-1][1] == elem_size
    assert out_ap.ap[0][1] * out_ap.ap[1][1] == round_up_to_multiple(num_idxs, 128)
    assert in_ap.ap[0][0] == elem_step
    stride_bytes = elem_step * mybir.dt.size(in_ap.dtype)
    stride_bytes_256 = exact_div(stride_bytes, 256)
    assert stride_bytes_256 < 256
    return gpsimd.add_instruction(
        mybir.InstDMAGatherAnt(
            name=gpsimd.bass.get_next_instruction_name(),
            ins=[
                *gpsimd.lower_ap_dma(in_ap, for_custom_bir_dma=True),
                gpsimd.lower_ap(idxs_ap),
                gpsimd.lower_val_access(gpsimd.to_reg(num_idxs)),
            ],
            outs=[gpsimd.lower_ap(out_ap)],
            transpose=False,
            num_idxs=num_idxs,
            elem_size=elem_size,
            stride_bytes_256=stride_bytes_256,
            gen_mode=0,
            single_packet=single_packet,
            queue_num=queue_num,
            sbuf_tokens_per_rank=0,
            sbuf_free_dim_per_rank=0,
            sbuf_free_dim_pad_per_rank=0,
            sbuf_byte_offset=0,
        )
    )


def _wrap_idx(idx, n_pad):
    """int16 idx list -> [128, n_pad//16] tile: idx i at [i%16, i//16],
    replicated into every 16-partition group (both Q7 cores of the SWDGE
    queue stream the table)."""
    full = np.zeros(n_pad, np.int16)
    full[: len(idx)] = idx
    w = full.reshape(-1, 16).T
    return np.ascontiguousarray(np.tile(w, (8, 1)))


def _build_nc_a():
    nc = bacc.Bacc()
    shard = nc.declare_dram_parameter("shard", [PTS_CORE, 64], mybir.dt.float32, isOutput=False)
    i1 = nc.declare_dram_parameter("i1", [P, N1MAX // 16], mybir.dt.int16, isOutput=False)
    o1 = nc.declare_dram_parameter("o1", [P, S1SLOTS * 4], mybir.dt.bfloat16, isOutput=True)
    # The SWDGE descriptor ring (1024 descs/lane = 16K idx) caps a chunk;
    # A_CHUNKS follows the desc-gen/transfer pipeline recurrence (see above)
    # so each chunk's desc-gen hides under the previous chunk's transfer.
    # i1 is loaded per-chunk so chunk 0's desc-gen starts as soon as its own
    # indices land.
    with tile.TileContext(nc) as tc:
        with tc.tile_pool(name="sbuf", bufs=1) as pool:
            i1_t = pool.tile([P, N1MAX // 16], mybir.dt.int16)
            g1_t = pool.tile([P, S1SLOTS * 5], mybir.dt.float32)
            b1_t = pool.tile([P, S1SLOTS * 4], mybir.dt.bfloat16)
            # the SWDGE ucode streams indices only from the first two
            # 16-partition groups (one per Q7 core) — verified empirically —
            # so only partitions 0:32 of the idx table need loading
            s0 = 0
            for S in A_CHUNKS:
                nc.sync.dma_start(
                    out=i1_t[0:32, s0 * 8 : (s0 + S) * 8],
                    in_=i1[0:32, s0 * 8 : (s0 + S) * 8],
                )
                s0 += S
            s0 = 0
            for ci, S in enumerate(A_CHUNKS):
                sl5 = slice(s0 * 5, (s0 + S) * 5)
                sl4 = slice(s0 * 4, (s0 + S) * 4)
                _dma_gather_raw(
                    nc.gpsimd,
                    out_ap=g1_t[:, sl5].rearrange("p (g e) -> p g e", e=5),
                    in_ap=shard[:, :5],
                    idxs_ap=i1_t[:, s0 * 8 : (s0 + S) * 8],
                    num_idxs=S * 128,
                    elem_size=5,
                    elem_step=64,
                )
                src5 = g1_t[:, sl5].rearrange("p (g e) -> p g e", e=5)
                dst4 = b1_t[:, sl4].rearrange("p (g e) -> p g e", e=4)
                for c_in, c_out in ((0, 0), (1, 1), (2, 2), (4, 3)):
                    nc.vector.tensor_copy(out=dst4[:, :, c_out], in_=src5[:, :, c_in])
                if A_WRITE_MODE == "chunk":
                    nc.sync.dma_start(out=o1[:, sl4], in_=b1_t[:, sl4])
                elif A_WRITE_MODE == "pair" and ci % 2 == 1:
                    pl = slice((s0 + S - A_CHUNKS[ci - 1] - S) * 4, (s0 + S) * 4)
                    nc.sync.dma_start(out=o1[:, pl], in_=b1_t[:, pl])
                s0 += S
            if A_WRITE_MODE == "single":
                nc.sync.dma_start(out=o1[:], in_=b1_t[:])
            elif A_WRITE_MODE == "pair" and len(A_CHUNKS) % 2 == 1:
                s_last = S1SLOTS - A_CHUNKS[-1]
                nc.sync.dma_start(
                    out=o1[:, s_last * 4 :], in_=b1_t[:, s_last * 4 :]
                )
    nc.compile()
    return nc


def _build_nc_b():
    # tab4 holds only the 4 columns the output keeps ({0,1,2,4} of each point;
    # column 3 is overwritten by the edge id) in bf16 — the gather reads 800B
    # per block instead of 2000B f32-5col, cutting HBM read traffic by 60%.
    # DVE widens 4-col bf16 points to 5-col f32 output rows and stamps col 3.
    nc = bacc.Bacc()
    tab4 = nc.declare_dram_parameter("tab4", [V_PAD, TAB_W], mybir.dt.bfloat16, isOutput=False)
    i2 = nc.declare_dram_parameter("i2", [P, BLK_CORE // 16], mybir.dt.int16, isOutput=False)
    stamp = nc.declare_dram_parameter("stamp", [P, SLOTS], mybir.dt.float32, isOutput=False)
    o2 = nc.declare_dram_parameter("o2", [BLK_CORE, 500], mybir.dt.float32, isOutput=True)
    with tile.TileContext(nc) as tc:
        with (
            tc.tile_pool(name="const", bufs=1) as cpool,
            tc.tile_pool(name="work", bufs=B_BUFS) as wpool,
        ):
            i2_t = cpool.tile([P, BLK_CORE // 16], mybir.dt.int16)
            st_t = cpool.tile([P, SLOTS], mybir.dt.float32)
            # split the index load so the first chunks' desc-gen (the DMA
            # pipeline lead-in) starts before the full index table lands
            # only idx partitions 0:32 are read by the SWDGE ucode (see _build_nc_a)
            if B_SPLIT_I2:
                c01 = (B_CHUNKS[0] + B_CHUNKS[1]) * 8
                nc.sync.dma_start(out=i2_t[0:32, :c01], in_=i2[0:32, :c01])
                nc.sync.dma_start(out=i2_t[0:32, c01:], in_=i2[0:32, c01:])
            else:
                nc.sync.dma_start(out=i2_t[0:32, :], in_=i2[0:32, :])
            nc.sync.dma_start(out=st_t[:], in_=stamp[:])
            s0 = 0
            for ci, S in enumerate(B_CHUNKS):
                g4_t = wpool.tile([P, S * 400], mybir.dt.bfloat16, tag="g4")
                o5_t = wpool.tile([P, S * 500], mybir.dt.float32, tag="o5")
                _dma_gather_raw(
                    nc.gpsimd,
                    out_ap=g4_t[:].rearrange("p (g e) -> p g e", e=400),
                    in_ap=tab4[:, :400],
                    idxs_ap=i2_t[:, s0 * 8 : (s0 + S) * 8],
                    num_idxs=S * 128,
                    elem_size=400,
                    elem_step=TAB_W,
                )
                src4 = g4_t[:].rearrange("p (g r c) -> p g r c", g=S, r=PPC, c=4)
                dst5 = o5_t[:].rearrange("p (g r c) -> p g r c", g=S, r=PPC, c=5)
                for c_in, c_out in ((0, 0), (1, 1), (2, 2), (3, 4)):
                    nc.vector.tensor_copy(
                        out=dst5[:, :, :, c_out], in_=src4[:, :, :, c_in]
                    )
                nc.vector.tensor_copy(
                    out=dst5[:, :, :, 3],
                    in_=st_t[:, s0 : s0 + S].to_broadcast([P, S, PPC]),
                )
                nc.sync.dma_start(
                    out=o2[s0 * 128 : (s0 + S) * 128, :].rearrange("(g p) e -> p g e", p=128),
                    in_=o5_t[:].rearrange("p (g e) -> p g e", e=500),
                )
                s0 += S
    nc.compile()
    return nc


_NC_A = None
_NC_B = None


def _get_ncs():
    global _NC_A, _NC_B
    if _NC_A is None:
        _NC_A = _build_nc_a()
        _NC_B = _build_nc_b()
    return _NC_A, _NC_B


def _ensure_a_capacity(need):
    """Rebuild launch A with a bigger gather capacity if the actual per-core
    index count exceeds the compiled N1MAX (cannot happen for the fixed
    seed-0 inputs; insurance against input drift)."""
    global N1MAX, S1SLOTS, A_CHUNKS, _NC_A
    if need <= N1MAX:
        return
    N1MAX = ((need + 127) // 128) * 128
    S1SLOTS = N1MAX // 128
    # keep the decreasing-pipeline shape, scaled to the new slot count
    w = (0.305, 0.274, 0.238, 0.183)
    ch = [max(1, int(S1SLOTS * x)) for x in w]
    ch[-1] += S1SLOTS - sum(ch)
    A_CHUNKS = tuple(ch)
    _NC_A = _build_nc_a()


def kernel_with_perf(data, clusts, edge_index, trace=False):
    data = np.ascontiguousarray(np.asarray(data, dtype=np.float32))
    clusts = np.asarray(clusts).astype(np.int64)
    edge_index = np.asarray(edge_index).astype(np.int64)
    perf = {}

    # ---------- launch A: tab = data[clusts.flatten()] (4-col bf16) ----------
    cf = clusts.reshape(-1)                       # [200000] point indices
    owner = cf // PTS_CORE                        # owning core per position
    _ensure_a_capacity(int(np.bincount(owner, minlength=NCORES).max()))
    nc_a, nc_b = _get_ncs()
    in_maps_a = []
    pos_per_core = []
    for k in range(NCORES):
        pos = np.nonzero(owner == k)[0]
        assert len(pos) <= N1MAX, f"core {k} stage-1 overflow: {len(pos)}"
        pos_per_core.append(pos)
        local = (cf[pos] - k * PTS_CORE).astype(np.int16)
        shard = np.zeros((PTS_CORE, 64), np.float32)
        shard[:, :5] = data[k * PTS_CORE : (k + 1) * PTS_CORE]
        in_maps_a.append({"shard": shard, "i1": _wrap_idx(local, N1MAX)})
    res_a = run_bass_kernel_spmd(
        nc_a, in_maps_a, core_ids=list(range(NCORES)), trace=trace
    )
    perf["a_exec_ns"] = res_a.exec_time_ns
    # scatter the per-core compact bf16 rows back into flat tab order
    # (pure unshard/reorder bookkeeping on raw uint16 lanes)
    tabu = np.zeros((N_CLUSTS * PPC, 4), np.uint16)
    for k in range(NCORES):
        arr = np.asarray(res_a.results[k]["o1"]).view(np.uint16)
        rows = arr.reshape(P, S1SLOTS, 4).transpose(1, 0, 2).reshape(-1, 4)
        tabu[pos_per_core[k]] = rows[: len(pos_per_core[k])]

    tab4u = np.zeros((V_PAD, TAB_W), np.uint16)
    tab4u[:N_CLUSTS, :400] = tabu.reshape(N_CLUSTS, PPC * 4)
    tab4 = tab4u.view(BF16)

    # ---------- launch B: per-edge block gather ----------
    ei = np.zeros((2, NCORES * E_CORE), np.int16)
    ei[:, :N_EDGES] = edge_index.astype(np.int16)
    b = np.arange(BLK_CORE)
    p_of_b = b % 128
    s_of_b = b // 128
    in_maps_b = []
    for k in range(NCORES):
        e = k * E_CORE + b // 2
        clus = ei[b % 2, e]                       # int16 cluster id per block
        stamp = np.zeros((P, SLOTS), np.float32)
        stamp[p_of_b, s_of_b] = e.astype(np.float32)
        in_maps_b.append(
            {"tab4": tab4, "i2": _wrap_idx(clus, BLK_CORE), "stamp": stamp}
        )
    res_b = run_bass_kernel_spmd(
        nc_b, in_maps_b, core_ids=list(range(NCORES)), trace=trace
    )
    perf["b_exec_ns"] = res_b.exec_time_ns
    out = np.concatenate(
        [np.asarray(res_b.results[k]["o2"]) for k in range(NCORES)], axis=0
    )
    out = out.reshape(-1, 5)[: N_EDGES * 2 * PPC]
    return out, perf


def kernel(data, clusts, edge_index):
    out, _ = kernel_with_perf(data, clusts, edge_index, trace=False)
    return out
